# revision 32
# baseline (speedup 1.0000x reference)
"""Trainium2 Bass kernel for nn_LocalizedFiltering (fused cat-conv2d x2 + residual + RMSNorm).

Strategy: sequence-parallel across 8 NeuronCores (one sequence of 2048 tokens +
1 cache row per core) -- no collectives needed. Matmuls run in bf16 (fp32 PSUM
accumulation); residual + RMSNorm in fp32.

Layout plan (keeps the PE array 100% on matmuls -- no on-chip transposes):
  Phase A (layer 1), feature-major: psum[feat, tok] = sum_k W1_k^T @ xT windows.
    Output features land on partitions -> per-partition bias add via the
    activation engine while copying psum -> xt2 (bf16), which is exactly the
    feature-major (lhsT) layout phase B needs.
  Phase B (layer 2), token-major: psum[tok, feat] = sum_k xt2_k^T @ W2 windows.
    The kernel-2 causal shift becomes a +-1 column offset of the xt2 lhsT
    window. Tokens land on partitions, so residual + bias (vector add with
    host-precomputed xres = x + b2) and RMSNorm (per-partition rstd) follow
    directly, and rows DMA straight out -- no transposes anywhere.
ln_weight is applied exactly on the host (out *= ln_weight).
"""

import os

import numpy as np
import ml_dtypes

BS, L, D, CACHE = 8, 2048, 2048, 64
T = BS * L
H = D // 2          # 1024
EPS = 1e-6
NCORES = 8
BLK = 512           # token block (= one PSUM bank of fp32)
NBLK = L // BLK     # 4
KT1 = D // 128      # 16 contraction tiles, layer 1
KT2 = H // 128      # 8 contraction tiles, layer 2
QT1 = H // 128      # 8 output-feature tiles, layer 1 (per half)
NTT = L // 128      # 16 token tiles, layer 2
FS = 512            # feature slice, layer 2 output
NFS = D // FS       # 4

TRACE = bool(int(os.environ.get("BASS_KERNEL_TRACE", "0")))
WINMAJ = 3          # win-major matmul order for block-0 rows k < WINMAJ
WSPLIT = 4          # W1 rows k < WSPLIT are DMA'd as two column halves
LAST_EXEC_NS = None
LAST_RESULTS = None

_NC_CACHE = {}


def _build_bass():
    if "nc" in _NC_CACHE:
        return _NC_CACHE["nc"]

    import concourse.bacc as bacc
    import concourse.tile as tile
    import concourse.mybir as mybir

    fp32 = mybir.dt.float32
    bf16 = mybir.dt.bfloat16
    Act = mybir.ActivationFunctionType

    nc = bacc.Bacc("TRN2", target_bir_lowering=False)

    xt1 = nc.declare_dram_parameter("xt1", [D, L + 1], bf16, isOutput=False)
    xres = nc.declare_dram_parameter("xres", [L, D], bf16, isOutput=False)
    c2 = nc.declare_dram_parameter("c2", [H, 1], bf16, isOutput=False)
    w1 = nc.declare_dram_parameter("w1", [D, D], bf16, isOutput=False)
    w2 = nc.declare_dram_parameter("w2", [H, 2 * D], bf16, isOutput=False)
    b1 = nc.declare_dram_parameter("b1", [H, 1], fp32, isOutput=False)
    out = nc.declare_dram_parameter("out", [L, D], bf16, isOutput=True)

    with tile.TileContext(nc) as tc, \
            tc.tile_pool(name="wpool", bufs=1) as wpool, \
            tc.tile_pool(name="wpre", bufs=1) as wpre, \
            tc.tile_pool(name="xt1p", bufs=2) as xt1p, \
            tc.tile_pool(name="xt2p", bufs=1) as xt2p, \
            tc.tile_pool(name="xresp", bufs=2) as xresp, \
            tc.tile_pool(name="rowp", bufs=3) as rowp, \
            tc.tile_pool(name="obp", bufs=1) as obp, \
            tc.tile_pool(name="scr", bufs=1) as scr, \
            tc.tile_pool(name="tmp", bufs=2) as tmp, \
            tc.tile_pool(name="const", bufs=1) as const, \
            tc.tile_pool(name="psp", bufs=8, space="PSUM") as psp:

        epssb = const.tile([128, 1], fp32)
        nc.vector.memset(epssb, EPS)

        # startup: the k=0 stationaries live in dedicated tiles so the first
        # matmuls gate on small DMAs -- wfirst (33KB) for the very first one,
        # then the win-0 / win-1 column halves of W1 row 0 as separate tiles
        # (dependency tracking is per-tile, so a half-row tile unblocks as
        # soon as its own DMA lands).
        wfirst = const.tile([128, 128], bf16, name="wfirst")
        nc.sync.dma_start(out=wfirst, in_=w1[0:128, 0:128])
        wA = const.tile([128, H], bf16, name="w1k0_a")
        wB = const.tile([128, H], bf16, name="w1k0_b")

        b1sb = const.tile([128, QT1, 1], fp32)
        xt2sb = xt2p.tile([128, KT2, L + 1], bf16)

        # ---------------- Phase A: layer 1 -> xt2 (bf16, feature-major) -----
        # W1 as 8 pair-tiles [128, 2, D]; the same slots are later reused by
        # the W2 k-tiles. Issue order interleaves weight rows with x tiles so
        # the k-outer matmul stream is never starved at startup.
        NW = KT1 // 2  # 8
        w1t = []
        x1k0 = []

        for j in range(NW):
            wj = wpool.tile([128, 2, D], bf16, tag=f"w{j}", name=f"w1_{j}")
            w1t.append(wj)
        for k in range(KT1):
            xk = xt1p.tile([128, BLK + 1], bf16, tag=f"x1k{k}", name=f"x1_0_{k}")
            nc.sync.dma_start(out=xk, in_=xt1[k * 128:(k + 1) * 128, 0:BLK + 1])
            if k == 0:
                nc.sync.dma_start(out=wA, in_=w1[0:128, 0:H])
                nc.sync.dma_start(out=wB, in_=w1[0:128, H:D])
            elif k < WSPLIT:
                # half-split the early rows: the win-0 matmuls of row k gate
                # on the first half's DMA, not the whole 512KB row.
                nc.sync.dma_start(
                    out=w1t[k // 2][:, k % 2, 0:H],
                    in_=w1[k * 128:(k + 1) * 128, 0:H])
                nc.sync.dma_start(
                    out=w1t[k // 2][:, k % 2, H:D],
                    in_=w1[k * 128:(k + 1) * 128, H:D])
            else:
                nc.sync.dma_start(
                    out=w1t[k // 2][:, k % 2, :],
                    in_=w1[k * 128:(k + 1) * 128, :])
            if k == 1:
                nc.sync.dma_start(
                    out=b1sb, in_=b1.rearrange("(q p) o -> p q o", p=128))
                nc.sync.dma_start(
                    out=xt2sb[:, :, 0:1], in_=c2.rearrange("(k p) o -> p k o", p=128))
            x1k0.append(xk)

        for b in range(NBLK):
            if b == 0:
                x1k = x1k0
            else:
                x1k = []
                for k in range(KT1):
                    xk = xt1p.tile([128, BLK + 1], bf16, tag=f"x1k{k}",
                                   name=f"x1_{b}_{k}")
                    nc.sync.dma_start(
                        out=xk,
                        in_=xt1[k * 128:(k + 1) * 128, b * BLK:b * BLK + BLK + 1])
                    x1k.append(xk)
            psA = [psp.tile([128, BLK], fp32, tag="mm", name=f"psA_{b}_{q}")
                   for q in range(QT1)]
            # k-outer over 8 concurrent psum banks; the final k round is
            # per-q (matmuls then the act drain) so banks free one by one and
            # the next block / phase B never waits on a bulk drain.
            for k in range(KT1):
                last = (k == KT1 - 1)
                if k == 0 or (b == 0 and k < WINMAJ):
                    # win-major in the startup wire race: the 8 win-0 matmuls
                    # of row k gate on the row's first-half DMA only, so the
                    # PE streams while the second half is still on the wire.
                    for q in range(QT1):
                        if k == 0:
                            lhs0 = wfirst if (b == 0 and q == 0) \
                                else wA[:, q * 128:(q + 1) * 128]
                        else:
                            lhs0 = w1t[k // 2][:, k % 2, q * 128:(q + 1) * 128]
                        nc.tensor.matmul(
                            psA[q], lhsT=lhs0, rhs=x1k[k][:, 0:BLK],
                            start=(k == 0), stop=False)
                    for q in range(QT1):
                        lhs1 = wB[:, q * 128:(q + 1) * 128] if k == 0 \
                            else w1t[k // 2][:, k % 2, H + q * 128:H + (q + 1) * 128]
                        nc.tensor.matmul(
                            psA[q], lhsT=lhs1,
                            rhs=x1k[k][:, 1:BLK + 1],
                            start=False, stop=False)
                    continue
                for q in range(QT1):
                    nc.tensor.matmul(
                        psA[q], lhsT=w1t[k // 2][:, k % 2, q * 128:(q + 1) * 128],
                        rhs=x1k[k][:, 0:BLK],
                        start=False, stop=False)
                    nc.tensor.matmul(
                        psA[q],
                        lhsT=w1t[k // 2][:, k % 2, H + q * 128:H + (q + 1) * 128],
                        rhs=x1k[k][:, 1:BLK + 1],
                        start=False, stop=last)
                    if last:
                        nc.scalar.activation(
                            out=xt2sb[:, q, 1 + b * BLK:1 + (b + 1) * BLK],
                            in_=psA[q],
                            func=Act.Identity, bias=b1sb[:, q, :], scale=1.0)

        # ---------------- Phase B: layer 2 + residual + RMSNorm -------------
        # token-major: psum[tok, feat]; lhsT = xt2 column windows (the causal
        # shift), rhs = W2 feature slices. W2 k=0,1 in dedicated slots
        # (prefetched during phase A); k>=2 reuse the W1 slots.
        w2t = []
        for k in range(KT2):
            if k < 2:
                wk = wpre.tile([128, 2 * D], bf16, tag=f"wp{k}", name=f"w2_{k}")
            else:
                wk = wpool.tile([128, 2 * D], bf16, tag=f"w{k - 2}", name=f"w2_{k}")
            nc.sync.dma_start(out=wk, in_=w2[k * 128:(k + 1) * 128, :])
            w2t.append(wk)

        for j in range(NTT):
            tok0 = j * 128
            # the last tile drains with finer feature slices: a shorter
            # add/square chain between its final matmul and the out DMA.
            fs = FS // 2 if j == NTT - 1 else FS
            nfs = D // fs
            xr = xresp.tile([128, D], bf16, tag="xres", name=f"xres_{j}")
            nc.sync.dma_start(out=xr, in_=xres[tok0:tok0 + 128, :])
            rowc = rowp.tile([128, D], fp32, tag="rowc", name=f"rowc_{j}")
            ob = obp.tile([128, D], bf16, tag="ob", name=f"ob_{j}")
            acc = tmp.tile([128, nfs], fp32, tag="acc", name=f"acc_{j}")
            for q in range(nfs):
                sl = slice(q * fs, (q + 1) * fs)
                ps = psp.tile([128, fs], fp32, tag="mm", name=f"psB_{j}_{q}")
                for k in range(KT2):
                    nc.tensor.matmul(
                        ps, lhsT=xt2sb[:, k, tok0:tok0 + 128],
                        rhs=w2t[k][:, q * fs:(q + 1) * fs],
                        start=(k == 0), stop=False)
                    nc.tensor.matmul(
                        ps, lhsT=xt2sb[:, k, tok0 + 1:tok0 + 129],
                        rhs=w2t[k][:, D + q * fs:D + (q + 1) * fs],
                        start=False, stop=(k == KT2 - 1))
                # o3 slice = o2 + (x + b2); then partial sum-of-squares so
                # almost no norm work remains after the last matmul.
                nc.vector.tensor_add(out=rowc[:, sl], in0=ps, in1=xr[:, sl])
                sq = scr.tile([128, fs], bf16, tag="sq", name=f"sq_{j}_{q}")
                nc.scalar.activation(
                    out=sq, in_=rowc[:, sl],
                    func=Act.Square, accum_out=acc[:, q:q + 1])
            rstd = tmp.tile([128, 1], fp32, tag="rstd", name=f"rstd_{j}")
            nc.vector.tensor_reduce(
                out=rstd, in_=acc, axis=mybir.AxisListType.X,
                op=mybir.AluOpType.add)
            nc.scalar.activation(
                out=rstd, in_=rstd, func=Act.Sqrt, bias=epssb, scale=1.0 / D)
            nc.vector.reciprocal(out=rstd, in_=rstd)
            for q in range(nfs):
                sl = slice(q * fs, (q + 1) * fs)
                # all scales on DVE: they queue right behind the reciprocal
                # with no cross-engine hop, and DVE is 2x throughput for the
                # bf16 destination.
                nc.vector.tensor_scalar_mul(
                    out=ob[:, sl], in0=rowc[:, sl], scalar1=rstd)
                if (q + 1) % (nfs // 2) == 0:
                    h0 = (q + 1 - nfs // 2) * fs
                    nc.sync.dma_start(
                        out=out[tok0:tok0 + 128, h0:(q + 1) * fs],
                        in_=ob[:, h0:(q + 1) * fs])

    nc.finalize()
    _NC_CACHE["nc"] = nc
    return nc


def _np_reference(inputs, pre_lf_indexs, out_lf_indexs, input_lf_loc, out_lf_loc,
                  inputs_loc, outputs_loc, lf1_caches, lf2_caches,
                  conv1_weight, conv2_weight, conv1_bias, conv2_bias, ln_weight):
    """Generic numpy fallback (only used if the index structure is unexpected)."""
    def fused(x, cache, pre_idx, in_lf_loc, in_loc, out_loc, W):
        bs = pre_idx.shape[0]
        xt = np.zeros((x.shape[0] + bs, x.shape[1]), x.dtype)
        xt[in_loc] = x
        xt[in_lf_loc] = cache[pre_idx]
        c = xt @ W
        h = c.shape[1] // 2
        y = c[:-1, :h] + c[1:, h:]
        return y[out_loc]

    o1 = fused(inputs, lf1_caches, pre_lf_indexs, input_lf_loc,
               inputs_loc, outputs_loc, conv1_weight) + conv1_bias
    o2 = fused(o1, lf2_caches, pre_lf_indexs, input_lf_loc,
               inputs_loc, outputs_loc, conv2_weight) + conv2_bias
    o3 = o2 + inputs
    var = np.mean(o3 * o3, axis=-1, keepdims=True)
    return (o3 / np.sqrt(var + EPS) * ln_weight).astype(np.float32)


def kernel(**inputs):
    global LAST_EXEC_NS, LAST_RESULTS
    inp = {k: np.asarray(v) for k, v in inputs.items()}
    x = inp["inputs"].astype(np.float32, copy=False)
    lnw = inp["ln_weight"].astype(np.float32, copy=False)

    s = np.arange(BS, dtype=np.int64)
    j = np.arange(L, dtype=np.int64)
    structured = (
        np.array_equal(inp["inputs_loc"], (s[:, None] * (L + 1) + 1 + j[None, :]).reshape(-1))
        and np.array_equal(inp["outputs_loc"], (s[:, None] * (L + 1) + j[None, :]).reshape(-1))
        and np.array_equal(inp["input_lf_loc"], s * (L + 1))
    )
    if not structured:
        return _np_reference(**inp)

    from concourse.bass_utils import run_bass_kernel_spmd

    nc = _build_bass()

    bf16 = ml_dtypes.bfloat16
    pre_idx = inp["pre_lf_indexs"].astype(np.int64)
    w1b = np.ascontiguousarray(inp["conv1_weight"].astype(bf16))
    w2b = np.ascontiguousarray(inp["conv2_weight"].astype(bf16))
    b1f = np.ascontiguousarray(inp["conv1_bias"].astype(np.float32).reshape(H, 1))
    b2f = inp["conv2_bias"].astype(np.float32)

    in_maps = []
    for sq in range(BS):
        xs = x[sq * L:(sq + 1) * L]                       # [2048, 2048]
        a = np.empty((D, L + 1), np.float32)
        a[:, 0] = inp["lf1_caches"][pre_idx[sq]]
        a[:, 1:] = xs.T
        in_maps.append({
            "xt1": np.ascontiguousarray(a.astype(bf16)),
            "xres": np.ascontiguousarray((xs + b2f[None, :]).astype(bf16)),
            "c2": np.ascontiguousarray(
                inp["lf2_caches"][pre_idx[sq]].astype(bf16).reshape(H, 1)),
            "w1": w1b,
            "w2": w2b,
            "b1": b1f,
        })

    res = run_bass_kernel_spmd(nc, in_maps, list(range(NCORES)), trace=TRACE)
    LAST_EXEC_NS = res.exec_time_ns
    LAST_RESULTS = res
    out = np.concatenate(
        [res.results[i]["out"].astype(np.float32) for i in range(NCORES)], axis=0)
    if not np.all(lnw == 1.0):
        out = out * lnw[None, :]
    return out.astype(np.float32)


# revision 41
# speedup vs baseline: 1.1178x; 1.1178x over previous
"""Trainium2 Bass kernel for nn_LocalizedFiltering (fused cat-conv2d x2 + residual + RMSNorm).

Strategy: sequence-parallel across 8 NeuronCores (one sequence of 2048 tokens +
1 cache row per core) -- no collectives needed.

Layer 1 runs in fp8-e4m3 DoubleRow mode: every matmul carries TWO contraction
k-tiles (the DoubleRow groups are adjacent k-row pairs -- plain strided APs),
streaming 256 contraction rows in the time bf16 streams 512. Accuracy is
recovered with hi+lo e4m3 pairs on both operands (x ~ xh+xl, W ~ Wh+Wl,
power-of-two pre-scales) accumulating the three significant products
xh*Wh + xl*Wh + xh*Wl in fp32 PSUM. Layer 2 stays bf16: its causal shift
lives on the lhsT (weight-load) side, whose fp8 ISA path requires aligned
strides/offsets that a +-1 token window cannot satisfy.

Layout (no on-chip transposes):
  Phase A (layer 1), feature-major: psum[feat, tok]; act drain descales, adds
    b1, emits xt2 bf16 -- exactly the lhsT layout phase B needs.
  Phase B (layer 2), token-major bf16: psum[tok, feat]; residual + bias via
    host-folded xres = x + b2; RMSNorm on token partitions; direct DMA out.
ln_weight is applied exactly on the host (out *= ln_weight).
"""

import os

import numpy as np
import ml_dtypes

BS, L, D, CACHE = 8, 2048, 2048, 64
T = BS * L
H = D // 2          # 1024
EPS = 1e-6
NCORES = 8
BLK = 512           # token block (= one PSUM bank of fp32)
NBLK = L // BLK     # 4
KP1 = D // 256      # 8 contraction k-PAIRS, layer 1
KP2 = H // 256      # 4 contraction k-pairs, layer 2
KT2 = H // 128      # 8 contraction tiles, layer 2 (bf16 phase B)
QT1 = H // 128      # 8 output-feature tiles, layer 1 (per half)
NTT = L // 128      # 16 token tiles, layer 2
FS = 512            # feature slice, layer 2 output

# power-of-two quantization scales (inputs ~N(0,1), weights ~N(0,0.02))
SX = 32.0           # layer-1 input scale
S2 = 32.0           # layer-2 input (o1) scale
SW1 = 2048.0
SW2 = 2048.0

TRACE = bool(int(os.environ.get("BASS_KERNEL_TRACE", "0")))
LAST_EXEC_NS = None
LAST_RESULTS = None

_NC_CACHE = {}


def _build_bass():
    if "nc" in _NC_CACHE:
        return _NC_CACHE["nc"]

    import concourse.bacc as bacc
    import concourse.tile as tile
    import concourse.mybir as mybir

    fp32 = mybir.dt.float32
    bf16 = mybir.dt.bfloat16
    f8 = mybir.dt.float8e4
    Act = mybir.ActivationFunctionType
    DR = mybir.MatmulPerfMode.DoubleRow

    nc = bacc.Bacc("TRN2", target_bir_lowering=False)

    x1h = nc.declare_dram_parameter("x1h", [D, L + 1], f8, isOutput=False)
    x1l = nc.declare_dram_parameter("x1l", [D, L + 1], f8, isOutput=False)
    xres = nc.declare_dram_parameter("xres", [L, D], bf16, isOutput=False)
    c2 = nc.declare_dram_parameter("c2", [H, 1], bf16, isOutput=False)
    w1h = nc.declare_dram_parameter("w1h", [D, D], f8, isOutput=False)
    w1l = nc.declare_dram_parameter("w1l", [D, D], f8, isOutput=False)
    w2 = nc.declare_dram_parameter("w2", [H, 2 * D], bf16, isOutput=False)
    b1s = nc.declare_dram_parameter("b1s", [H, 1], fp32, isOutput=False)
    out = nc.declare_dram_parameter("out", [L, D], bf16, isOutput=True)

    with tile.TileContext(nc) as tc, \
            tc.tile_pool(name="wpool", bufs=1) as wpool, \
            tc.tile_pool(name="x1p", bufs=2) as x1p, \
            tc.tile_pool(name="xt2p", bufs=1) as xt2p, \
            tc.tile_pool(name="wpre", bufs=1) as wpre, \
            tc.tile_pool(name="xresp", bufs=2) as xresp, \
            tc.tile_pool(name="rowp", bufs=3) as rowp, \
            tc.tile_pool(name="obp", bufs=1) as obp, \
            tc.tile_pool(name="scr", bufs=1) as scr, \
            tc.tile_pool(name="tmp", bufs=2) as tmp, \
            tc.tile_pool(name="const", bufs=1) as const, \
            tc.tile_pool(name="psp", bufs=8, space="PSUM") as psp:

        epssb = const.tile([128, 1], fp32)
        nc.vector.memset(epssb, EPS)

        # startup tiles for k-pair 0 (hi parts gate the first instructions)
        wfirst = const.tile([128, 2, 128], f8, name="wfirst")
        nc.sync.dma_start(
            out=wfirst,
            in_=w1h[0:256, 0:128].rearrange("(i p) c -> p i c", p=128))
        wAh = const.tile([128, 2, H], f8, name="wAh")
        wBh = const.tile([128, 2, H], f8, name="wBh")
        wAl = const.tile([128, 2, H], f8, name="wAl")
        wBl = const.tile([128, 2, H], f8, name="wBl")

        b1sb = const.tile([128, QT1, 1], fp32)
        xt2sb = xt2p.tile([128, KP2 * 2, L + 1], bf16)

        # ---------------- Phase A: layer 1 -> xt2 hi/lo fp8 -----------------
        # W1 as 4+4 quad tiles [128, 4, D] (hi and lo); each DoubleRow lhsT is
        # a k-row PAIR [128, 2, 128] sliced from a quad. The same 8 slots are
        # later reused by the W2 pair tiles [128, 2, 2D].
        NQ = KP1 // 2  # 4 quads
        w1hq, w1lq = [], []
        for j in range(NQ):
            w1hq.append(wpool.tile([128, 4, D], f8, tag=f"w{j}", name=f"w1h_{j}"))
            w1lq.append(wpool.tile([128, 4, D], f8, tag=f"w{4 + j}", name=f"w1l_{j}"))

        x1kh0, x1kl0 = [], []

        def x1dma(dst, src, kp, b):
            c0 = b * BLK
            nc.sync.dma_start(
                out=dst,
                in_=src[kp * 256:(kp + 1) * 256, c0:c0 + BLK + 1].rearrange(
                    "(i p) f -> p i f", p=128))

        for kp in range(KP1):
            xh = x1p.tile([128, 2, BLK + 1], f8, tag=f"xh{kp}", name=f"x1h_0_{kp}")
            x1dma(xh, x1h, kp, 0)
            x1kh0.append(xh)
            if kp == 0:
                nc.sync.dma_start(
                    out=wAh, in_=w1h[0:256, 0:H].rearrange("(i p) c -> p i c", p=128))
                nc.sync.dma_start(
                    out=wBh, in_=w1h[0:256, H:D].rearrange("(i p) c -> p i c", p=128))
            xl = x1p.tile([128, 2, BLK + 1], f8, tag=f"xl{kp}", name=f"x1l_0_{kp}")
            x1dma(xl, x1l, kp, 0)
            x1kl0.append(xl)
            if kp == 0:
                nc.sync.dma_start(
                    out=wAl, in_=w1l[0:256, 0:H].rearrange("(i p) c -> p i c", p=128))
                nc.sync.dma_start(
                    out=wBl, in_=w1l[0:256, H:D].rearrange("(i p) c -> p i c", p=128))
                nc.sync.dma_start(
                    out=b1sb, in_=b1s.rearrange("(q p) o -> p q o", p=128))
                nc.sync.dma_start(
                    out=xt2sb[:, :, 0:1], in_=c2.rearrange("(k p) o -> p k o", p=128))
            elif kp % 2 == 1:
                j = kp // 2
                nc.sync.dma_start(
                    out=w1hq[j],
                    in_=w1h[j * 512:(j + 1) * 512, :].rearrange(
                        "(i p) c -> p i c", p=128))
                nc.sync.dma_start(
                    out=w1lq[j],
                    in_=w1l[j * 512:(j + 1) * 512, :].rearrange(
                        "(i p) c -> p i c", p=128))

        def lhsA(kp, win, q, part, b):
            """[128, 2, 128] k-pair weight view for window win, q-tile q."""
            if kp == 0:
                if part == 'h':
                    if b == 0 and win == 0 and q == 0:
                        return wfirst
                    t = wAh if win == 0 else wBh
                else:
                    t = wAl if win == 0 else wBl
                return t[:, :, q * 128:(q + 1) * 128]
            quad = (w1hq if part == 'h' else w1lq)[kp // 2]
            s = 2 * (kp % 2)
            c0 = win * H + q * 128
            return quad[:, s:s + 2, c0:c0 + 128]

        for b in range(NBLK):
            if b == 0:
                x1kh, x1kl = x1kh0, x1kl0
            else:
                x1kh, x1kl = [], []
                for kp in range(KP1):
                    xh = x1p.tile([128, 2, BLK + 1], f8, tag=f"xh{kp}",
                                  name=f"x1h_{b}_{kp}")
                    x1dma(xh, x1h, kp, b)
                    x1kh.append(xh)
                    xl = x1p.tile([128, 2, BLK + 1], f8, tag=f"xl{kp}",
                                  name=f"x1l_{b}_{kp}")
                    x1dma(xl, x1l, kp, b)
                    x1kl.append(xl)
            psA = [psp.tile([128, BLK], fp32, tag="mm", name=f"psA_{b}_{q}")
                   for q in range(QT1)]

            def mmA(q, kp, win, p, start, stop, b=b, x1kh=x1kh, x1kl=x1kl,
                    psA=psA):
                wpart = 'h' if p in ('hh', 'lh') else 'l'
                xt = x1kh[kp] if p in ('hh', 'hl') else x1kl[kp]
                nc.tensor.matmul(
                    psA[q], lhsT=lhsA(kp, win, q, wpart, b),
                    rhs=xt[:, :, win:win + BLK],
                    start=start, stop=stop, perf_mode=DR)

            for kp in range(KP1):
                last = (kp == KP1 - 1)
                if not last:
                    for win in (0, 1):
                        for p in ('hh', 'lh', 'hl'):
                            for q in range(QT1):
                                mmA(q, kp, win, p,
                                    start=(kp == 0 and win == 0 and p == 'hh'),
                                    stop=False)
                    continue
                # final k-pair: per-q (matmuls then drain) so PSUM banks free
                # one at a time and the next block / phase B never stalls.
                for q in range(QT1):
                    for win in (0, 1):
                        for p in ('hh', 'lh', 'hl'):
                            mmA(q, kp, win, p, start=False,
                                stop=(win == 1 and p == 'hl'))
                    # drain: xt2 = psum/(SX*SW1) + b1, emitted bf16 for the
                    # bf16 phase B.
                    nc.scalar.activation(
                        out=xt2sb[:, q, 1 + b * BLK:1 + (b + 1) * BLK],
                        in_=psA[q], func=Act.Identity,
                        bias=b1sb[:, q, :], scale=1.0 / (SX * SW1))

        # ---------------- Phase B: layer 2 + residual + RMSNorm -------------
        # token-major: psum[tok, feat]; lhsT = xt2 column windows (the causal
        # shift), rhs = W2 feature slices. W2 k=0,1 in dedicated slots
        # (prefetched during phase A); k>=2 reuse the W1 slots.
        w2t = []
        for k in range(KT2):
            if k < 2:
                wk = wpre.tile([128, 2 * D], bf16, tag=f"wp{k}", name=f"w2_{k}")
            else:
                wk = wpool.tile([128, 2 * D], bf16, tag=f"w{k - 2}", name=f"w2_{k}")
            nc.sync.dma_start(out=wk, in_=w2[k * 128:(k + 1) * 128, :])
            w2t.append(wk)

        for j in range(NTT):
            tok0 = j * 128
            # the last tile drains with finer feature slices: a shorter
            # add/square chain between its final matmul and the out DMA.
            fs = FS // 2 if j == NTT - 1 else FS
            nfs = D // fs
            xr = xresp.tile([128, D], bf16, tag="xres", name=f"xres_{j}")
            nc.sync.dma_start(out=xr, in_=xres[tok0:tok0 + 128, :])
            rowc = rowp.tile([128, D], fp32, tag="rowc", name=f"rowc_{j}")
            ob = obp.tile([128, D], bf16, tag="ob", name=f"ob_{j}")
            acc = tmp.tile([128, nfs], fp32, tag="acc", name=f"acc_{j}")
            for q in range(nfs):
                sl = slice(q * fs, (q + 1) * fs)
                ps = psp.tile([128, fs], fp32, tag="mm", name=f"psB_{j}_{q}")
                for k in range(KT2):
                    nc.tensor.matmul(
                        ps, lhsT=xt2sb[:, k, tok0:tok0 + 128],
                        rhs=w2t[k][:, q * fs:(q + 1) * fs],
                        start=(k == 0), stop=False)
                    nc.tensor.matmul(
                        ps, lhsT=xt2sb[:, k, tok0 + 1:tok0 + 129],
                        rhs=w2t[k][:, D + q * fs:D + (q + 1) * fs],
                        start=False, stop=(k == KT2 - 1))
                # o3 slice = o2 + (x + b2); then partial sum-of-squares so
                # almost no norm work remains after the last matmul.
                nc.vector.tensor_add(out=rowc[:, sl], in0=ps, in1=xr[:, sl])
                sq = scr.tile([128, fs], bf16, tag="sq", name=f"sq_{j}_{q}")
                nc.scalar.activation(
                    out=sq, in_=rowc[:, sl],
                    func=Act.Square, accum_out=acc[:, q:q + 1])
            rstd = tmp.tile([128, 1], fp32, tag="rstd", name=f"rstd_{j}")
            nc.vector.tensor_reduce(
                out=rstd, in_=acc, axis=mybir.AxisListType.X,
                op=mybir.AluOpType.add)
            nc.scalar.activation(
                out=rstd, in_=rstd, func=Act.Sqrt, bias=epssb, scale=1.0 / D)
            nc.vector.reciprocal(out=rstd, in_=rstd)
            for q in range(nfs):
                sl = slice(q * fs, (q + 1) * fs)
                # all scales on DVE: they queue right behind the reciprocal
                # with no cross-engine hop, and DVE is 2x throughput for the
                # bf16 destination.
                nc.vector.tensor_scalar_mul(
                    out=ob[:, sl], in0=rowc[:, sl], scalar1=rstd)
                if (q + 1) % (nfs // 2) == 0:
                    h0 = (q + 1 - nfs // 2) * fs
                    nc.sync.dma_start(
                        out=out[tok0:tok0 + 128, h0:(q + 1) * fs],
                        in_=ob[:, h0:(q + 1) * fs])

    nc.finalize()
    _NC_CACHE["nc"] = nc
    return nc


def _np_reference(inputs, pre_lf_indexs, out_lf_indexs, input_lf_loc, out_lf_loc,
                  inputs_loc, outputs_loc, lf1_caches, lf2_caches,
                  conv1_weight, conv2_weight, conv1_bias, conv2_bias, ln_weight):
    """Generic numpy fallback (only used if the index structure is unexpected)."""
    def fused(x, cache, pre_idx, in_lf_loc, in_loc, out_loc, W):
        bs = pre_idx.shape[0]
        xt = np.zeros((x.shape[0] + bs, x.shape[1]), x.dtype)
        xt[in_loc] = x
        xt[in_lf_loc] = cache[pre_idx]
        c = xt @ W
        h = c.shape[1] // 2
        y = c[:-1, :h] + c[1:, h:]
        return y[out_loc]

    o1 = fused(inputs, lf1_caches, pre_lf_indexs, input_lf_loc,
               inputs_loc, outputs_loc, conv1_weight) + conv1_bias
    o2 = fused(o1, lf2_caches, pre_lf_indexs, input_lf_loc,
               inputs_loc, outputs_loc, conv2_weight) + conv2_bias
    o3 = o2 + inputs
    var = np.mean(o3 * o3, axis=-1, keepdims=True)
    return (o3 / np.sqrt(var + EPS) * ln_weight).astype(np.float32)


def _split8(v, s):
    """hi/lo e4m3 pair representing v*s."""
    e4 = ml_dtypes.float8_e4m3
    hi = np.ascontiguousarray((v * s).astype(e4))
    lo = np.ascontiguousarray((v * s - hi.astype(np.float32)).astype(e4))
    return hi, lo


def kernel(**inputs):
    global LAST_EXEC_NS, LAST_RESULTS
    inp = {k: np.asarray(v) for k, v in inputs.items()}
    x = inp["inputs"].astype(np.float32, copy=False)
    lnw = inp["ln_weight"].astype(np.float32, copy=False)

    s = np.arange(BS, dtype=np.int64)
    j = np.arange(L, dtype=np.int64)
    structured = (
        np.array_equal(inp["inputs_loc"], (s[:, None] * (L + 1) + 1 + j[None, :]).reshape(-1))
        and np.array_equal(inp["outputs_loc"], (s[:, None] * (L + 1) + j[None, :]).reshape(-1))
        and np.array_equal(inp["input_lf_loc"], s * (L + 1))
    )
    if not structured:
        return _np_reference(**inp)

    from concourse.bass_utils import run_bass_kernel_spmd

    nc = _build_bass()

    bf16 = ml_dtypes.bfloat16
    pre_idx = inp["pre_lf_indexs"].astype(np.int64)
    w1hb, w1lb = _split8(inp["conv1_weight"].astype(np.float32), SW1)
    w2b = np.ascontiguousarray(inp["conv2_weight"].astype(bf16))
    b1f = np.ascontiguousarray(inp["conv1_bias"].astype(np.float32).reshape(H, 1))
    b2f = inp["conv2_bias"].astype(np.float32)

    in_maps = []
    for sq in range(BS):
        xs = x[sq * L:(sq + 1) * L]                       # [2048, 2048]
        a = np.empty((D, L + 1), np.float32)
        a[:, 0] = inp["lf1_caches"][pre_idx[sq]]
        a[:, 1:] = xs.T
        xh, xl = _split8(a, SX)
        in_maps.append({
            "x1h": xh, "x1l": xl,
            "xres": np.ascontiguousarray((xs + b2f[None, :]).astype(bf16)),
            "c2": np.ascontiguousarray(
                inp["lf2_caches"][pre_idx[sq]].astype(bf16).reshape(H, 1)),
            "w1h": w1hb, "w1l": w1lb,
            "w2": w2b,
            "b1s": b1f,
        })

    res = run_bass_kernel_spmd(nc, in_maps, list(range(NCORES)), trace=TRACE)
    LAST_EXEC_NS = res.exec_time_ns
    LAST_RESULTS = res
    out = np.concatenate(
        [res.results[i]["out"].astype(np.float32) for i in range(NCORES)], axis=0)
    if not np.all(lnw == 1.0):
        out = out * lnw[None, :]
    return out.astype(np.float32)


# revision 42
# speedup vs baseline: 1.1311x; 1.0119x over previous
"""Trainium2 Bass kernel for nn_LocalizedFiltering (fused cat-conv2d x2 + residual + RMSNorm).

Strategy: sequence-parallel across 8 NeuronCores (one sequence of 2048 tokens +
1 cache row per core) -- no collectives needed.

Layer 1 runs in fp8-e4m3 DoubleRow mode: every matmul carries TWO contraction
k-tiles (the DoubleRow groups are adjacent k-row pairs -- plain strided APs),
streaming 256 contraction rows in the time bf16 streams 512. Accuracy is
recovered with hi+lo e4m3 pairs on both operands (x ~ xh+xl, W ~ Wh+Wl,
power-of-two pre-scales) accumulating the three significant products
xh*Wh + xl*Wh + xh*Wl in fp32 PSUM. Layer 2 stays bf16: its causal shift
lives on the lhsT (weight-load) side, whose fp8 ISA path requires aligned
strides/offsets that a +-1 token window cannot satisfy.

Layout (no on-chip transposes):
  Phase A (layer 1), feature-major: psum[feat, tok]; act drain descales, adds
    b1, emits xt2 bf16 -- exactly the lhsT layout phase B needs.
  Phase B (layer 2), token-major bf16: psum[tok, feat]; residual + bias via
    host-folded xres = x + b2; RMSNorm on token partitions; direct DMA out.
ln_weight is applied exactly on the host (out *= ln_weight).
"""

import os

import numpy as np
import ml_dtypes

BS, L, D, CACHE = 8, 2048, 2048, 64
T = BS * L
H = D // 2          # 1024
EPS = 1e-6
NCORES = 8
BLK = 512           # token block (= one PSUM bank of fp32)
NBLK = L // BLK     # 4
KP1 = D // 256      # 8 contraction k-PAIRS, layer 1
KP2 = H // 256      # 4 contraction k-pairs, layer 2
KT2 = H // 128      # 8 contraction tiles, layer 2 (bf16 phase B)
QT1 = H // 128      # 8 output-feature tiles, layer 1 (per half)
NTT = L // 128      # 16 token tiles, layer 2
FS = 512            # feature slice, layer 2 output

# power-of-two quantization scales (inputs ~N(0,1), weights ~N(0,0.02))
SX = 32.0           # layer-1 input scale
S2 = 32.0           # layer-2 input (o1) scale
SW1 = 2048.0
SW2 = 2048.0

TRACE = bool(int(os.environ.get("BASS_KERNEL_TRACE", "0")))
LAST_EXEC_NS = None
LAST_RESULTS = None

_NC_CACHE = {}


def _build_bass():
    if "nc" in _NC_CACHE:
        return _NC_CACHE["nc"]

    import concourse.bacc as bacc
    import concourse.tile as tile
    import concourse.mybir as mybir

    fp32 = mybir.dt.float32
    bf16 = mybir.dt.bfloat16
    f8 = mybir.dt.float8e4
    Act = mybir.ActivationFunctionType
    DR = mybir.MatmulPerfMode.DoubleRow

    nc = bacc.Bacc("TRN2", target_bir_lowering=False)

    x1h = nc.declare_dram_parameter("x1h", [D, L + 1], f8, isOutput=False)
    x1l = nc.declare_dram_parameter("x1l", [D, L + 1], f8, isOutput=False)
    xres = nc.declare_dram_parameter("xres", [L, D], bf16, isOutput=False)
    c2 = nc.declare_dram_parameter("c2", [H, 1], bf16, isOutput=False)
    w1h = nc.declare_dram_parameter("w1h", [D, D], f8, isOutput=False)
    w1l = nc.declare_dram_parameter("w1l", [D, D], f8, isOutput=False)
    w2 = nc.declare_dram_parameter("w2", [H, 2 * D], bf16, isOutput=False)
    b1s = nc.declare_dram_parameter("b1s", [H, 1], fp32, isOutput=False)
    out = nc.declare_dram_parameter("out", [L, D], bf16, isOutput=True)

    with tile.TileContext(nc) as tc, \
            tc.tile_pool(name="wpool", bufs=1) as wpool, \
            tc.tile_pool(name="x1p", bufs=2) as x1p, \
            tc.tile_pool(name="xt2p", bufs=1) as xt2p, \
            tc.tile_pool(name="wpre", bufs=1) as wpre, \
            tc.tile_pool(name="xresp", bufs=2) as xresp, \
            tc.tile_pool(name="rowp", bufs=3) as rowp, \
            tc.tile_pool(name="obp", bufs=1) as obp, \
            tc.tile_pool(name="scr", bufs=1) as scr, \
            tc.tile_pool(name="tmp", bufs=2) as tmp, \
            tc.tile_pool(name="const", bufs=1) as const, \
            tc.tile_pool(name="psp", bufs=8, space="PSUM") as psp:

        epssb = const.tile([128, 1], fp32)
        nc.vector.memset(epssb, EPS)

        # startup tiles for k-pair 0 (hi parts gate the first instructions)
        wfirst = const.tile([128, 2, 128], f8, name="wfirst")
        nc.sync.dma_start(
            out=wfirst,
            in_=w1h[0:256, 0:128].rearrange("(i p) c -> p i c", p=128))
        wAh = const.tile([128, 2, H], f8, name="wAh")
        wBh = const.tile([128, 2, H], f8, name="wBh")
        wAl = const.tile([128, 2, H], f8, name="wAl")
        wBl = const.tile([128, 2, H], f8, name="wBl")

        b1sb = const.tile([128, QT1, 1], fp32)
        xt2sb = xt2p.tile([128, KP2 * 2, L + 1], bf16)

        # ---------------- Phase A: layer 1 -> xt2 hi/lo fp8 -----------------
        # W1 as 4+4 quad tiles [128, 4, D] (hi and lo); each DoubleRow lhsT is
        # a k-row PAIR [128, 2, 128] sliced from a quad. The same 8 slots are
        # later reused by the W2 pair tiles [128, 2, 2D].
        NQ = KP1 // 2  # 4 quads
        w1hq, w1lq = [], []
        for j in range(NQ):
            w1hq.append(wpool.tile([128, 4, D], f8, tag=f"w{j}", name=f"w1h_{j}"))
            w1lq.append(wpool.tile([128, 4, D], f8, tag=f"w{4 + j}", name=f"w1l_{j}"))

        x1kh0, x1kl0 = [], []

        def x1dma(dst, src, kp, b):
            c0 = b * BLK
            nc.sync.dma_start(
                out=dst,
                in_=src[kp * 256:(kp + 1) * 256, c0:c0 + BLK + 1].rearrange(
                    "(i p) f -> p i f", p=128))

        for kp in range(KP1):
            xh = x1p.tile([128, 2, BLK + 1], f8, tag=f"xh{kp}", name=f"x1h_0_{kp}")
            x1dma(xh, x1h, kp, 0)
            x1kh0.append(xh)
            if kp == 0:
                nc.sync.dma_start(
                    out=wAh, in_=w1h[0:256, 0:H].rearrange("(i p) c -> p i c", p=128))
                nc.sync.dma_start(
                    out=wBh, in_=w1h[0:256, H:D].rearrange("(i p) c -> p i c", p=128))
            xl = x1p.tile([128, 2, BLK + 1], f8, tag=f"xl{kp}", name=f"x1l_0_{kp}")
            x1dma(xl, x1l, kp, 0)
            x1kl0.append(xl)
            if kp == 0:
                nc.sync.dma_start(
                    out=wAl, in_=w1l[0:256, 0:H].rearrange("(i p) c -> p i c", p=128))
                nc.sync.dma_start(
                    out=wBl, in_=w1l[0:256, H:D].rearrange("(i p) c -> p i c", p=128))
                nc.sync.dma_start(
                    out=b1sb, in_=b1s.rearrange("(q p) o -> p q o", p=128))
                nc.sync.dma_start(
                    out=xt2sb[:, :, 0:1], in_=c2.rearrange("(k p) o -> p k o", p=128))
            else:
                # per-pair halves of the quad tiles, issued alongside their
                # k-pair's x tiles so the weight stream never falls behind.
                j, s = kp // 2, 2 * (kp % 2)
                nc.sync.dma_start(
                    out=w1hq[j][:, s:s + 2, :],
                    in_=w1h[kp * 256:(kp + 1) * 256, :].rearrange(
                        "(i p) c -> p i c", p=128))
                nc.sync.dma_start(
                    out=w1lq[j][:, s:s + 2, :],
                    in_=w1l[kp * 256:(kp + 1) * 256, :].rearrange(
                        "(i p) c -> p i c", p=128))

        def lhsA(kp, win, q, part, b):
            """[128, 2, 128] k-pair weight view for window win, q-tile q."""
            if kp == 0:
                if part == 'h':
                    if b == 0 and win == 0 and q == 0:
                        return wfirst
                    t = wAh if win == 0 else wBh
                else:
                    t = wAl if win == 0 else wBl
                return t[:, :, q * 128:(q + 1) * 128]
            quad = (w1hq if part == 'h' else w1lq)[kp // 2]
            s = 2 * (kp % 2)
            c0 = win * H + q * 128
            return quad[:, s:s + 2, c0:c0 + 128]

        for b in range(NBLK):
            if b == 0:
                x1kh, x1kl = x1kh0, x1kl0
            else:
                x1kh, x1kl = [], []
                for kp in range(KP1):
                    xh = x1p.tile([128, 2, BLK + 1], f8, tag=f"xh{kp}",
                                  name=f"x1h_{b}_{kp}")
                    x1dma(xh, x1h, kp, b)
                    x1kh.append(xh)
                    xl = x1p.tile([128, 2, BLK + 1], f8, tag=f"xl{kp}",
                                  name=f"x1l_{b}_{kp}")
                    x1dma(xl, x1l, kp, b)
                    x1kl.append(xl)
            psA = [psp.tile([128, BLK], fp32, tag="mm", name=f"psA_{b}_{q}")
                   for q in range(QT1)]

            def mmA(q, kp, win, p, start, stop, b=b, x1kh=x1kh, x1kl=x1kl,
                    psA=psA):
                wpart = 'h' if p in ('hh', 'lh') else 'l'
                xt = x1kh[kp] if p in ('hh', 'hl') else x1kl[kp]
                nc.tensor.matmul(
                    psA[q], lhsT=lhsA(kp, win, q, wpart, b),
                    rhs=xt[:, :, win:win + BLK],
                    start=start, stop=stop, perf_mode=DR)

            for kp in range(KP1):
                last = (kp == KP1 - 1)
                if not last:
                    for win in (0, 1):
                        for p in ('hh', 'lh', 'hl'):
                            for q in range(QT1):
                                mmA(q, kp, win, p,
                                    start=(kp == 0 and win == 0 and p == 'hh'),
                                    stop=False)
                    continue
                # final k-pair: per-q (matmuls then drain) so PSUM banks free
                # one at a time and the next block / phase B never stalls.
                for q in range(QT1):
                    for win in (0, 1):
                        for p in ('hh', 'lh', 'hl'):
                            mmA(q, kp, win, p, start=False,
                                stop=(win == 1 and p == 'hl'))
                    # drain: xt2 = psum/(SX*SW1) + b1, emitted bf16 for the
                    # bf16 phase B.
                    nc.scalar.activation(
                        out=xt2sb[:, q, 1 + b * BLK:1 + (b + 1) * BLK],
                        in_=psA[q], func=Act.Identity,
                        bias=b1sb[:, q, :], scale=1.0 / (SX * SW1))

        # ---------------- Phase B: layer 2 + residual + RMSNorm -------------
        # token-major: psum[tok, feat]; lhsT = xt2 column windows (the causal
        # shift), rhs = W2 feature slices. W2 k=0,1 in dedicated slots
        # (prefetched during phase A); k>=2 reuse the W1 slots.
        w2t = []
        for k in range(KT2):
            if k < 2:
                wk = wpre.tile([128, 2 * D], bf16, tag=f"wp{k}", name=f"w2_{k}")
            else:
                wk = wpool.tile([128, 2 * D], bf16, tag=f"w{k - 2}", name=f"w2_{k}")
            nc.sync.dma_start(out=wk, in_=w2[k * 128:(k + 1) * 128, :])
            w2t.append(wk)

        for j in range(NTT):
            tok0 = j * 128
            # the last tile drains with finer feature slices: a shorter
            # add/square chain between its final matmul and the out DMA.
            fs = FS // 2 if j == NTT - 1 else FS
            nfs = D // fs
            xr = xresp.tile([128, D], bf16, tag="xres", name=f"xres_{j}")
            nc.sync.dma_start(out=xr, in_=xres[tok0:tok0 + 128, :])
            rowc = rowp.tile([128, D], fp32, tag="rowc", name=f"rowc_{j}")
            ob = obp.tile([128, D], bf16, tag="ob", name=f"ob_{j}")
            acc = tmp.tile([128, nfs], fp32, tag="acc", name=f"acc_{j}")
            for q in range(nfs):
                sl = slice(q * fs, (q + 1) * fs)
                ps = psp.tile([128, fs], fp32, tag="mm", name=f"psB_{j}_{q}")
                for k in range(KT2):
                    nc.tensor.matmul(
                        ps, lhsT=xt2sb[:, k, tok0:tok0 + 128],
                        rhs=w2t[k][:, q * fs:(q + 1) * fs],
                        start=(k == 0), stop=False)
                    nc.tensor.matmul(
                        ps, lhsT=xt2sb[:, k, tok0 + 1:tok0 + 129],
                        rhs=w2t[k][:, D + q * fs:D + (q + 1) * fs],
                        start=False, stop=(k == KT2 - 1))
                # o3 slice = o2 + (x + b2); then partial sum-of-squares so
                # almost no norm work remains after the last matmul.
                nc.vector.tensor_add(out=rowc[:, sl], in0=ps, in1=xr[:, sl])
                sq = scr.tile([128, fs], bf16, tag="sq", name=f"sq_{j}_{q}")
                nc.scalar.activation(
                    out=sq, in_=rowc[:, sl],
                    func=Act.Square, accum_out=acc[:, q:q + 1])
            rstd = tmp.tile([128, 1], fp32, tag="rstd", name=f"rstd_{j}")
            nc.vector.tensor_reduce(
                out=rstd, in_=acc, axis=mybir.AxisListType.X,
                op=mybir.AluOpType.add)
            nc.scalar.activation(
                out=rstd, in_=rstd, func=Act.Sqrt, bias=epssb, scale=1.0 / D)
            nc.vector.reciprocal(out=rstd, in_=rstd)
            for q in range(nfs):
                sl = slice(q * fs, (q + 1) * fs)
                # all scales on DVE: they queue right behind the reciprocal
                # with no cross-engine hop, and DVE is 2x throughput for the
                # bf16 destination.
                nc.vector.tensor_scalar_mul(
                    out=ob[:, sl], in0=rowc[:, sl], scalar1=rstd)
                if (q + 1) % (nfs // 2) == 0:
                    h0 = (q + 1 - nfs // 2) * fs
                    nc.sync.dma_start(
                        out=out[tok0:tok0 + 128, h0:(q + 1) * fs],
                        in_=ob[:, h0:(q + 1) * fs])

    nc.finalize()
    _NC_CACHE["nc"] = nc
    return nc


def _np_reference(inputs, pre_lf_indexs, out_lf_indexs, input_lf_loc, out_lf_loc,
                  inputs_loc, outputs_loc, lf1_caches, lf2_caches,
                  conv1_weight, conv2_weight, conv1_bias, conv2_bias, ln_weight):
    """Generic numpy fallback (only used if the index structure is unexpected)."""
    def fused(x, cache, pre_idx, in_lf_loc, in_loc, out_loc, W):
        bs = pre_idx.shape[0]
        xt = np.zeros((x.shape[0] + bs, x.shape[1]), x.dtype)
        xt[in_loc] = x
        xt[in_lf_loc] = cache[pre_idx]
        c = xt @ W
        h = c.shape[1] // 2
        y = c[:-1, :h] + c[1:, h:]
        return y[out_loc]

    o1 = fused(inputs, lf1_caches, pre_lf_indexs, input_lf_loc,
               inputs_loc, outputs_loc, conv1_weight) + conv1_bias
    o2 = fused(o1, lf2_caches, pre_lf_indexs, input_lf_loc,
               inputs_loc, outputs_loc, conv2_weight) + conv2_bias
    o3 = o2 + inputs
    var = np.mean(o3 * o3, axis=-1, keepdims=True)
    return (o3 / np.sqrt(var + EPS) * ln_weight).astype(np.float32)


def _split8(v, s):
    """hi/lo e4m3 pair representing v*s."""
    e4 = ml_dtypes.float8_e4m3
    hi = np.ascontiguousarray((v * s).astype(e4))
    lo = np.ascontiguousarray((v * s - hi.astype(np.float32)).astype(e4))
    return hi, lo


def kernel(**inputs):
    global LAST_EXEC_NS, LAST_RESULTS
    inp = {k: np.asarray(v) for k, v in inputs.items()}
    x = inp["inputs"].astype(np.float32, copy=False)
    lnw = inp["ln_weight"].astype(np.float32, copy=False)

    s = np.arange(BS, dtype=np.int64)
    j = np.arange(L, dtype=np.int64)
    structured = (
        np.array_equal(inp["inputs_loc"], (s[:, None] * (L + 1) + 1 + j[None, :]).reshape(-1))
        and np.array_equal(inp["outputs_loc"], (s[:, None] * (L + 1) + j[None, :]).reshape(-1))
        and np.array_equal(inp["input_lf_loc"], s * (L + 1))
    )
    if not structured:
        return _np_reference(**inp)

    from concourse.bass_utils import run_bass_kernel_spmd

    nc = _build_bass()

    bf16 = ml_dtypes.bfloat16
    pre_idx = inp["pre_lf_indexs"].astype(np.int64)
    w1hb, w1lb = _split8(inp["conv1_weight"].astype(np.float32), SW1)
    w2b = np.ascontiguousarray(inp["conv2_weight"].astype(bf16))
    b1f = np.ascontiguousarray(inp["conv1_bias"].astype(np.float32).reshape(H, 1))
    b2f = inp["conv2_bias"].astype(np.float32)

    in_maps = []
    for sq in range(BS):
        xs = x[sq * L:(sq + 1) * L]                       # [2048, 2048]
        a = np.empty((D, L + 1), np.float32)
        a[:, 0] = inp["lf1_caches"][pre_idx[sq]]
        a[:, 1:] = xs.T
        xh, xl = _split8(a, SX)
        in_maps.append({
            "x1h": xh, "x1l": xl,
            "xres": np.ascontiguousarray((xs + b2f[None, :]).astype(bf16)),
            "c2": np.ascontiguousarray(
                inp["lf2_caches"][pre_idx[sq]].astype(bf16).reshape(H, 1)),
            "w1h": w1hb, "w1l": w1lb,
            "w2": w2b,
            "b1s": b1f,
        })

    res = run_bass_kernel_spmd(nc, in_maps, list(range(NCORES)), trace=TRACE)
    LAST_EXEC_NS = res.exec_time_ns
    LAST_RESULTS = res
    out = np.concatenate(
        [res.results[i]["out"].astype(np.float32) for i in range(NCORES)], axis=0)
    if not np.all(lnw == 1.0):
        out = out * lnw[None, :]
    return out.astype(np.float32)


# revision 46
# speedup vs baseline: 1.3073x; 1.1557x over previous
"""Trainium2 Bass kernel for nn_LocalizedFiltering (fused cat-conv2d x2 + residual + RMSNorm).

Strategy: sequence-parallel across 8 NeuronCores (one sequence of 2048 tokens +
1 cache row per core) -- no collectives needed.

Layer 1 runs in fp8-e4m3 DoubleRow mode: every matmul carries TWO contraction
k-tiles (the DoubleRow groups are adjacent k-row pairs -- plain strided APs),
streaming 256 contraction rows in the time bf16 streams 512. Accuracy is
recovered with hi+lo e4m3 pairs on both operands (x ~ xh+xl, W ~ Wh+Wl,
power-of-two pre-scales) accumulating the three significant products
xh*Wh + xl*Wh + xh*Wl in fp32 PSUM. Layer 2 stays bf16: its causal shift
lives on the lhsT (weight-load) side, whose fp8 ISA path requires aligned
strides/offsets that a +-1 token window cannot satisfy.

Layout (no on-chip transposes):
  Phase A (layer 1), feature-major: psum[feat, tok]; act drain descales, adds
    b1, emits xt2 bf16 -- exactly the lhsT layout phase B needs.
  Phase B (layer 2), token-major bf16: psum[tok, feat]; residual + bias via
    host-folded xres = x + b2; RMSNorm on token partitions; direct DMA out.
ln_weight is applied exactly on the host (out *= ln_weight).
"""

import os

import numpy as np
import ml_dtypes

BS, L, D, CACHE = 8, 2048, 2048, 64
T = BS * L
H = D // 2          # 1024
EPS = 1e-6
NCORES = 8
BLK = 512           # token block (= one PSUM bank of fp32)
NBLK = L // BLK     # 4
KP1 = D // 256      # 8 contraction k-PAIRS, layer 1
KP2 = H // 256      # 4 contraction k-pairs, layer 2
KT2 = H // 128      # 8 contraction tiles, layer 2 (bf16 phase B)
QT1 = H // 128      # 8 output-feature tiles, layer 1 (per half)
NTT = L // 128      # 16 token tiles, layer 2
FS = 512            # feature slice, layer 2 output

# power-of-two quantization scales (inputs ~N(0,1), weights ~N(0,0.02))
SX = 32.0           # layer-1 input scale
S2 = 32.0           # layer-2 input (o1) scale
SW1 = 2048.0
SW2 = 2048.0
LP2 = L             # xt2 row length (win0 never reads col L; pow2 strides)
EPS_EFF = EPS * (S2 * SW2) ** 2   # folds the psum descale into RMSNorm

TRACE = bool(int(os.environ.get("BASS_KERNEL_TRACE", "0")))
LAST_EXEC_NS = None
LAST_RESULTS = None

_NC_CACHE = {}


def _build_bass():
    if "nc" in _NC_CACHE:
        return _NC_CACHE["nc"]

    import concourse.bacc as bacc
    import concourse.tile as tile
    import concourse.mybir as mybir

    fp32 = mybir.dt.float32
    bf16 = mybir.dt.bfloat16
    f8 = mybir.dt.float8e4
    Act = mybir.ActivationFunctionType
    DR = mybir.MatmulPerfMode.DoubleRow

    nc = bacc.Bacc("TRN2", target_bir_lowering=False)

    x1h = nc.declare_dram_parameter("x1h", [D, L + 1], f8, isOutput=False)
    x1l = nc.declare_dram_parameter("x1l", [D, L + 1], f8, isOutput=False)
    xres = nc.declare_dram_parameter("xres", [L, D], bf16, isOutput=False)
    c2h = nc.declare_dram_parameter("c2h", [H, 1], f8, isOutput=False)
    c2l = nc.declare_dram_parameter("c2l", [H, 1], f8, isOutput=False)
    w1h = nc.declare_dram_parameter("w1h", [D, D], f8, isOutput=False)
    w1l = nc.declare_dram_parameter("w1l", [D, D], f8, isOutput=False)
    w2h = nc.declare_dram_parameter("w2h", [H, 2 * D], f8, isOutput=False)
    w2l = nc.declare_dram_parameter("w2l", [H, 2 * D], f8, isOutput=False)
    b1s = nc.declare_dram_parameter("b1s", [H, 1], fp32, isOutput=False)
    out = nc.declare_dram_parameter("out", [L, D], bf16, isOutput=True)

    with tile.TileContext(nc) as tc, \
            tc.tile_pool(name="wpool", bufs=1) as wpool, \
            tc.tile_pool(name="x1p", bufs=2) as x1p, \
            tc.tile_pool(name="xt2p", bufs=1) as xt2p, \
            tc.tile_pool(name="t2p", bufs=8) as t2p, \
            tc.tile_pool(name="xresp", bufs=1) as xresp, \
            tc.tile_pool(name="rowp", bufs=2) as rowp, \
            tc.tile_pool(name="scr", bufs=1) as scr, \
            tc.tile_pool(name="tmp", bufs=2) as tmp, \
            tc.tile_pool(name="const", bufs=1) as const, \
            tc.tile_pool(name="psp", bufs=8, space="PSUM") as psp:

        epssb = const.tile([128, 1], fp32)
        nc.vector.memset(epssb, EPS_EFF)

        # startup tiles for k-pair 0 (hi parts gate the first instructions)
        wfirst = const.tile([128, 2, 128], f8, name="wfirst")
        nc.sync.dma_start(
            out=wfirst,
            in_=w1h[0:256, 0:128].rearrange("(i p) c -> p i c", p=128))
        b1sb = const.tile([128, QT1, 1], fp32)
        # xt2 hi/lo fp8 with BOTH window shifts materialized: [...] win=0 col t
        # holds xt2[t], win=1 col t holds xt2[t+1]; pair/win strides stay
        # 4-aligned so phase-B fp8 LDWEIGHTS APs are legal.
        xt2wh = xt2p.tile([128, KP2, 2, 2, LP2], f8)   # [kp, win, pair, col]
        xt2wl = xt2p.tile([128, KP2, 2, 2, LP2], f8)

        # ---------------- Phase A: layer 1 -> xt2 hi/lo fp8 -----------------
        # W1 as 4+4 quad tiles [128, 4, D] (hi and lo); each DoubleRow lhsT is
        # a k-row PAIR [128, 2, 128] sliced from a quad. The same 8 slots are
        # later reused by the W2 pair tiles [128, 2, 2D].
        NQ = KP1 // 2  # 4 quads
        w1hq, w1lq = [], []
        for j in range(NQ):
            w1hq.append(wpool.tile([128, 4, D], f8, tag=f"w{j}", name=f"w1h_{j}"))
            w1lq.append(wpool.tile([128, 4, D], f8, tag=f"w{4 + j}", name=f"w1l_{j}"))

        x1kh0, x1kl0 = [], []

        def x1dma(dst, src, kp, b):
            c0 = b * BLK
            nc.sync.dma_start(
                out=dst,
                in_=src[kp * 256:(kp + 1) * 256, c0:c0 + BLK + 1].rearrange(
                    "(i p) f -> p i f", p=128))

        for kp in range(KP1):
            xh = x1p.tile([128, 2, BLK + 1], f8, tag=f"xh{kp}", name=f"x1h_0_{kp}")
            x1dma(xh, x1h, kp, 0)
            x1kh0.append(xh)
            xl = x1p.tile([128, 2, BLK + 1], f8, tag=f"xl{kp}", name=f"x1l_0_{kp}")
            x1dma(xl, x1l, kp, 0)
            x1kl0.append(xl)
            # per-pair halves of the quad tiles, issued alongside their
            # k-pair's x tiles so the weight stream never falls behind.
            j, s = kp // 2, 2 * (kp % 2)
            nc.sync.dma_start(
                out=w1hq[j][:, s:s + 2, :],
                in_=w1h[kp * 256:(kp + 1) * 256, :].rearrange(
                    "(i p) c -> p i c", p=128))
            nc.sync.dma_start(
                out=w1lq[j][:, s:s + 2, :],
                in_=w1l[kp * 256:(kp + 1) * 256, :].rearrange(
                    "(i p) c -> p i c", p=128))
            if kp == 0:
                nc.sync.dma_start(
                    out=b1sb, in_=b1s.rearrange("(q p) o -> p q o", p=128))
                for pq in (0, 1):
                    nc.sync.dma_start(
                        out=xt2wh[:, :, 0, pq, 0:1],
                        in_=c2h.rearrange("(a i p) o -> p a i o",
                                          p=128, a=KP2)[:, :, pq, :])
                    nc.sync.dma_start(
                        out=xt2wl[:, :, 0, pq, 0:1],
                        in_=c2l.rearrange("(a i p) o -> p a i o",
                                          p=128, a=KP2)[:, :, pq, :])

        def lhsA(kp, win, q, part, b):
            """[128, 2, 128] k-pair weight view for window win, q-tile q."""
            if kp == 0 and part == 'h' and b == 0 and win == 0 and q == 0:
                return wfirst
            quad = (w1hq if part == 'h' else w1lq)[kp // 2]
            s = 2 * (kp % 2)
            c0 = win * H + q * 128
            return quad[:, s:s + 2, c0:c0 + 128]

        for b in range(NBLK):
            if b == 0:
                x1kh, x1kl = x1kh0, x1kl0
            else:
                x1kh, x1kl = [], []
                for kp in range(KP1):
                    xh = x1p.tile([128, 2, BLK + 1], f8, tag=f"xh{kp}",
                                  name=f"x1h_{b}_{kp}")
                    x1dma(xh, x1h, kp, b)
                    x1kh.append(xh)
                    xl = x1p.tile([128, 2, BLK + 1], f8, tag=f"xl{kp}",
                                  name=f"x1l_{b}_{kp}")
                    x1dma(xl, x1l, kp, b)
                    x1kl.append(xl)
            psA = [psp.tile([128, BLK], fp32, tag="mm", name=f"psA_{b}_{q}")
                   for q in range(QT1)]

            def mmA(q, kp, win, p, start, stop, b=b, x1kh=x1kh, x1kl=x1kl,
                    psA=psA):
                wpart = 'h' if p in ('hh', 'lh') else 'l'
                xt = x1kh[kp] if p in ('hh', 'hl') else x1kl[kp]
                nc.tensor.matmul(
                    psA[q], lhsT=lhsA(kp, win, q, wpart, b),
                    rhs=xt[:, :, win:win + BLK],
                    start=start, stop=stop, perf_mode=DR)

            for kp in range(KP1):
                last = (kp == KP1 - 1)
                if not last:
                    for win in (0, 1):
                        for p in ('hh', 'lh', 'hl'):
                            for q in range(QT1):
                                mmA(q, kp, win, p,
                                    start=(kp == 0 and win == 0 and p == 'hh'),
                                    stop=False)
                    continue
                # final k-pair: per-q matmuls then the bank-freeing t2 copy;
                # the hi/lo quantization chains run afterwards so all 8 PSUM
                # banks free at t2-copy rate for the next block.
                t2s = []
                for q in range(QT1):
                    for win in (0, 1):
                        for p in ('hh', 'lh', 'hl'):
                            mmA(q, kp, win, p, start=False,
                                stop=(win == 1 and p == 'hl'))
                    t2 = t2p.tile([128, BLK], fp32, tag="t2",
                                  name=f"t2_{b}_{q}")
                    nc.scalar.activation(
                        out=t2, in_=psA[q], func=Act.Identity,
                        bias=b1sb[:, q, :], scale=S2 / (SX * SW1))
                    t2s.append(t2)
                for q in range(QT1):
                    kq, pq = q // 2, q % 2
                    c0, c1 = 1 + b * BLK, 1 + (b + 1) * BLK
                    w0 = min(c1, LP2) - c0   # win0 never stores col L
                    hi0 = xt2wh[:, kq, 0, pq, c0:c0 + w0]
                    nc.scalar.activation(out=hi0, in_=t2s[q][:, 0:w0],
                                         func=Act.Identity)
                    hi1 = xt2wh[:, kq, 1, pq, c0 - 1:c1 - 1]
                    nc.scalar.activation(out=hi1, in_=t2s[q], func=Act.Identity)
                    nc.vector.tensor_sub(out=t2s[q], in0=t2s[q], in1=hi1)
                    nc.scalar.activation(
                        out=xt2wl[:, kq, 1, pq, c0 - 1:c1 - 1],
                        in_=t2s[q], func=Act.Identity)
                    nc.vector.tensor_copy(
                        out=xt2wl[:, kq, 0, pq, c0:c0 + w0],
                        in_=xt2wl[:, kq, 1, pq, c0 - 1:c0 - 1 + w0])

        # ---------------- Phase B: layer 2 + residual + RMSNorm -------------
        # token-major fp8 DoubleRow: lhsT = xt2 hi/lo k-row pairs from the
        # materialized window copies (aligned offsets), rhs = W2 hi/lo pair
        # tiles reusing the W1 quad slots. rowc/out stay bf16; the psum
        # descale folds into xres (host-scaled) and EPS_EFF.
        w2hp, w2lp = [], []
        for kp in range(KP2):
            wh = wpool.tile([128, 2, 2 * D], f8, tag=f"w{kp}", name=f"w2h_{kp}")
            nc.sync.dma_start(
                out=wh, in_=w2h[kp * 256:(kp + 1) * 256, :].rearrange(
                    "(i p) c -> p i c", p=128))
            w2hp.append(wh)
            wl = wpool.tile([128, 2, 2 * D], f8, tag=f"w{4 + kp}", name=f"w2l_{kp}")
            nc.sync.dma_start(
                out=wl, in_=w2l[kp * 256:(kp + 1) * 256, :].rearrange(
                    "(i p) c -> p i c", p=128))
            w2lp.append(wl)

        for j in range(NTT):
            tok0 = j * 128
            # the last tile drains with finer feature slices: a shorter
            # add/square chain between its final matmul and the out DMA.
            fs = FS // 2 if j == NTT - 1 else FS
            nfs = D // fs
            xr = xresp.tile([128, D], bf16, tag="xres", name=f"xres_{j}")
            nc.sync.dma_start(out=xr, in_=xres[tok0:tok0 + 128, :])
            rowc = rowp.tile([128, D], bf16, tag="rowc", name=f"rowc_{j}")
            acc = tmp.tile([128, nfs], fp32, tag="acc", name=f"acc_{j}")
            for q in range(nfs):
                sl = slice(q * fs, (q + 1) * fs)
                ps = psp.tile([128, fs], fp32, tag="mm", name=f"psB_{j}_{q}")
                first = True
                for kp in range(KP2):
                    for win in (0, 1):
                        c0 = win * D + q * fs
                        for p in ('hh', 'lh', 'hl'):
                            xt = xt2wh if p in ('hh', 'hl') else xt2wl
                            wt = (w2hp if p in ('hh', 'lh') else w2lp)[kp]
                            nc.tensor.matmul(
                                ps, lhsT=xt[:, kp, win, :, tok0:tok0 + 128],
                                rhs=wt[:, :, c0:c0 + fs],
                                start=first,
                                stop=(kp == KP2 - 1 and win == 1 and p == 'hl'),
                                perf_mode=DR)
                            first = False
                # rowc = S2*SW2*o3 slice (xres is pre-scaled on the host);
                # partial sum-of-squares right away.
                nc.vector.tensor_add(out=rowc[:, sl], in0=ps, in1=xr[:, sl])
                sq = scr.tile([128, fs], bf16, tag="sq", name=f"sq_{j}_{q}")
                nc.scalar.activation(
                    out=sq, in_=rowc[:, sl],
                    func=Act.Square, accum_out=acc[:, q:q + 1])
            # rstd_eff = 1/sqrt(S/D + EPS*(S2*SW2)^2) absorbs the descale, so
            # rowc * rstd_eff is the final normalized output.
            rstd = tmp.tile([128, 1], fp32, tag="rstd", name=f"rstd_{j}")
            nc.vector.tensor_reduce(
                out=rstd, in_=acc, axis=mybir.AxisListType.X,
                op=mybir.AluOpType.add)
            nc.scalar.activation(
                out=rstd, in_=rstd, func=Act.Sqrt, bias=epssb, scale=1.0 / D)
            nc.vector.reciprocal(out=rstd, in_=rstd)
            for q in range(nfs):
                sl = slice(q * fs, (q + 1) * fs)
                nc.vector.tensor_scalar_mul(
                    out=rowc[:, sl], in0=rowc[:, sl], scalar1=rstd)
                if (q + 1) % (nfs // 2) == 0:
                    h0 = (q + 1 - nfs // 2) * fs
                    nc.sync.dma_start(
                        out=out[tok0:tok0 + 128, h0:(q + 1) * fs],
                        in_=rowc[:, h0:(q + 1) * fs])

    nc.finalize()
    _NC_CACHE["nc"] = nc
    return nc


def _np_reference(inputs, pre_lf_indexs, out_lf_indexs, input_lf_loc, out_lf_loc,
                  inputs_loc, outputs_loc, lf1_caches, lf2_caches,
                  conv1_weight, conv2_weight, conv1_bias, conv2_bias, ln_weight):
    """Generic numpy fallback (only used if the index structure is unexpected)."""
    def fused(x, cache, pre_idx, in_lf_loc, in_loc, out_loc, W):
        bs = pre_idx.shape[0]
        xt = np.zeros((x.shape[0] + bs, x.shape[1]), x.dtype)
        xt[in_loc] = x
        xt[in_lf_loc] = cache[pre_idx]
        c = xt @ W
        h = c.shape[1] // 2
        y = c[:-1, :h] + c[1:, h:]
        return y[out_loc]

    o1 = fused(inputs, lf1_caches, pre_lf_indexs, input_lf_loc,
               inputs_loc, outputs_loc, conv1_weight) + conv1_bias
    o2 = fused(o1, lf2_caches, pre_lf_indexs, input_lf_loc,
               inputs_loc, outputs_loc, conv2_weight) + conv2_bias
    o3 = o2 + inputs
    var = np.mean(o3 * o3, axis=-1, keepdims=True)
    return (o3 / np.sqrt(var + EPS) * ln_weight).astype(np.float32)


def _split8(v, s):
    """hi/lo e4m3 pair representing v*s."""
    e4 = ml_dtypes.float8_e4m3
    hi = np.ascontiguousarray((v * s).astype(e4))
    lo = np.ascontiguousarray((v * s - hi.astype(np.float32)).astype(e4))
    return hi, lo


def kernel(**inputs):
    global LAST_EXEC_NS, LAST_RESULTS
    inp = {k: np.asarray(v) for k, v in inputs.items()}
    x = inp["inputs"].astype(np.float32, copy=False)
    lnw = inp["ln_weight"].astype(np.float32, copy=False)

    s = np.arange(BS, dtype=np.int64)
    j = np.arange(L, dtype=np.int64)
    structured = (
        np.array_equal(inp["inputs_loc"], (s[:, None] * (L + 1) + 1 + j[None, :]).reshape(-1))
        and np.array_equal(inp["outputs_loc"], (s[:, None] * (L + 1) + j[None, :]).reshape(-1))
        and np.array_equal(inp["input_lf_loc"], s * (L + 1))
    )
    if not structured:
        return _np_reference(**inp)

    from concourse.bass_utils import run_bass_kernel_spmd

    nc = _build_bass()

    bf16 = ml_dtypes.bfloat16
    pre_idx = inp["pre_lf_indexs"].astype(np.int64)
    w1hb, w1lb = _split8(inp["conv1_weight"].astype(np.float32), SW1)
    w2hb, w2lb = _split8(inp["conv2_weight"].astype(np.float32), SW2)
    b1f = np.ascontiguousarray(
        (inp["conv1_bias"].astype(np.float32) * S2).reshape(H, 1))
    b2f = inp["conv2_bias"].astype(np.float32)

    in_maps = []
    for sq in range(BS):
        xs = x[sq * L:(sq + 1) * L]                       # [2048, 2048]
        a = np.empty((D, L + 1), np.float32)
        a[:, 0] = inp["lf1_caches"][pre_idx[sq]]
        a[:, 1:] = xs.T
        xh, xl = _split8(a, SX)
        ch, cl = _split8(
            inp["lf2_caches"][pre_idx[sq]].astype(np.float32).reshape(H, 1), S2)
        in_maps.append({
            "x1h": xh, "x1l": xl,
            "xres": np.ascontiguousarray(
                ((xs + b2f[None, :]) * (S2 * SW2)).astype(bf16)),
            "c2h": ch, "c2l": cl,
            "w1h": w1hb, "w1l": w1lb,
            "w2h": w2hb, "w2l": w2lb,
            "b1s": b1f,
        })

    res = run_bass_kernel_spmd(nc, in_maps, list(range(NCORES)), trace=TRACE)
    LAST_EXEC_NS = res.exec_time_ns
    LAST_RESULTS = res
    out = np.concatenate(
        [res.results[i]["out"].astype(np.float32) for i in range(NCORES)], axis=0)
    if not np.all(lnw == 1.0):
        out = out * lnw[None, :]
    return out.astype(np.float32)


# revision 49
# speedup vs baseline: 1.3134x; 1.0047x over previous
"""Trainium2 Bass kernel for nn_LocalizedFiltering (fused cat-conv2d x2 + residual + RMSNorm).

Strategy: sequence-parallel across 8 NeuronCores (one sequence of 2048 tokens +
1 cache row per core) -- no collectives needed.

Layer 1 runs in fp8-e4m3 DoubleRow mode: every matmul carries TWO contraction
k-tiles (the DoubleRow groups are adjacent k-row pairs -- plain strided APs),
streaming 256 contraction rows in the time bf16 streams 512. Accuracy is
recovered with hi+lo e4m3 pairs on both operands (x ~ xh+xl, W ~ Wh+Wl,
power-of-two pre-scales) accumulating the three significant products
xh*Wh + xl*Wh + xh*Wl in fp32 PSUM. Layer 2 stays bf16: its causal shift
lives on the lhsT (weight-load) side, whose fp8 ISA path requires aligned
strides/offsets that a +-1 token window cannot satisfy.

Layout (no on-chip transposes):
  Phase A (layer 1), feature-major: psum[feat, tok]; act drain descales, adds
    b1, emits xt2 bf16 -- exactly the lhsT layout phase B needs.
  Phase B (layer 2), token-major bf16: psum[tok, feat]; residual + bias via
    host-folded xres = x + b2; RMSNorm on token partitions; direct DMA out.
ln_weight is applied exactly on the host (out *= ln_weight).
"""

import os

import numpy as np
import ml_dtypes

BS, L, D, CACHE = 8, 2048, 2048, 64
T = BS * L
H = D // 2          # 1024
EPS = 1e-6
NCORES = 8
BLK = 512           # token block (= one PSUM bank of fp32)
NBLK = L // BLK     # 4
KP1 = D // 256      # 8 contraction k-PAIRS, layer 1
KP2 = H // 256      # 4 contraction k-pairs, layer 2
KT2 = H // 128      # 8 contraction tiles, layer 2 (bf16 phase B)
QT1 = H // 128      # 8 output-feature tiles, layer 1 (per half)
NTT = L // 128      # 16 token tiles, layer 2
FS = 512            # feature slice, layer 2 output

# power-of-two quantization scales (inputs ~N(0,1), weights ~N(0,0.02))
SX = 32.0           # layer-1 input scale
S2 = 32.0           # layer-2 input (o1) scale
SW1 = 2048.0
SW2 = 2048.0
LP2 = L             # xt2 row length (win0 never reads col L; pow2 strides)
EPS_EFF = EPS * (S2 * SW2) ** 2   # folds the psum descale into RMSNorm

TRACE = bool(int(os.environ.get("BASS_KERNEL_TRACE", "0")))
LAST_EXEC_NS = None
LAST_RESULTS = None

_NC_CACHE = {}


def _build_bass():
    if "nc" in _NC_CACHE:
        return _NC_CACHE["nc"]

    import concourse.bacc as bacc
    import concourse.tile as tile
    import concourse.mybir as mybir

    fp32 = mybir.dt.float32
    bf16 = mybir.dt.bfloat16
    f8 = mybir.dt.float8e4
    Act = mybir.ActivationFunctionType
    DR = mybir.MatmulPerfMode.DoubleRow

    nc = bacc.Bacc("TRN2", target_bir_lowering=False)

    x1h = nc.declare_dram_parameter("x1h", [D, L + 1], f8, isOutput=False)
    x1l = nc.declare_dram_parameter("x1l", [D, L + 1], f8, isOutput=False)
    xres = nc.declare_dram_parameter("xres", [L, D], bf16, isOutput=False)
    c2h = nc.declare_dram_parameter("c2h", [H, 1], f8, isOutput=False)
    c2l = nc.declare_dram_parameter("c2l", [H, 1], f8, isOutput=False)
    w1h = nc.declare_dram_parameter("w1h", [D, D], f8, isOutput=False)
    w1l = nc.declare_dram_parameter("w1l", [D, D], f8, isOutput=False)
    w2h = nc.declare_dram_parameter("w2h", [H, 2 * D], f8, isOutput=False)
    w2l = nc.declare_dram_parameter("w2l", [H, 2 * D], f8, isOutput=False)
    b1s = nc.declare_dram_parameter("b1s", [H, 1], fp32, isOutput=False)
    out = nc.declare_dram_parameter("out", [L, D], bf16, isOutput=True)

    with tile.TileContext(nc) as tc, \
            tc.tile_pool(name="wpool", bufs=1) as wpool, \
            tc.tile_pool(name="x1p", bufs=2) as x1p, \
            tc.tile_pool(name="xt2p", bufs=1) as xt2p, \
            tc.tile_pool(name="t2p", bufs=8) as t2p, \
            tc.tile_pool(name="xresp", bufs=1) as xresp, \
            tc.tile_pool(name="rowp", bufs=2) as rowp, \
            tc.tile_pool(name="scr", bufs=1) as scr, \
            tc.tile_pool(name="tmp", bufs=2) as tmp, \
            tc.tile_pool(name="const", bufs=1) as const, \
            tc.tile_pool(name="psp", bufs=8, space="PSUM") as psp:

        epssb = const.tile([128, 1], fp32)
        nc.vector.memset(epssb, EPS_EFF)

        # startup tiles for k-pair 0 (hi parts gate the first instructions)
        wfirst = const.tile([128, 2, 128], f8, name="wfirst")
        nc.sync.dma_start(
            out=wfirst,
            in_=w1h[0:256, 0:128].rearrange("(i p) c -> p i c", p=128))
        wAh = const.tile([128, 2, H], f8, name="wAh")
        b1sb = const.tile([128, QT1, 1], fp32)
        # xt2 hi/lo fp8 with BOTH window shifts materialized: [...] win=0 col t
        # holds xt2[t], win=1 col t holds xt2[t+1]; pair/win strides stay
        # 4-aligned so phase-B fp8 LDWEIGHTS APs are legal.
        xt2wh = xt2p.tile([128, KP2, 2, 2, LP2], f8)   # [kp, win, pair, col]
        xt2wl = xt2p.tile([128, KP2, 2, 2, LP2], f8)

        # ---------------- Phase A: layer 1 -> xt2 hi/lo fp8 -----------------
        # W1 as 4+4 quad tiles [128, 4, D] (hi and lo); each DoubleRow lhsT is
        # a k-row PAIR [128, 2, 128] sliced from a quad. The same 8 slots are
        # later reused by the W2 pair tiles [128, 2, 2D].
        NQ = KP1 // 2  # 4 quads
        w1hq, w1lq = [], []
        for j in range(NQ):
            w1hq.append(wpool.tile([128, 4, D], f8, tag=f"w{j}", name=f"w1h_{j}"))
            w1lq.append(wpool.tile([128, 4, D], f8, tag=f"w{4 + j}", name=f"w1l_{j}"))

        x1kh0, x1kl0 = [], []

        def x1dma(dst, src, kp, b):
            c0 = b * BLK
            nc.sync.dma_start(
                out=dst,
                in_=src[kp * 256:(kp + 1) * 256, c0:c0 + BLK + 1].rearrange(
                    "(i p) f -> p i f", p=128))

        for kp in range(KP1):
            xh = x1p.tile([128, 2, BLK + 1], f8, tag=f"xh{kp}", name=f"x1h_0_{kp}")
            x1dma(xh, x1h, kp, 0)
            x1kh0.append(xh)
            if kp == 0:
                nc.sync.dma_start(
                    out=wAh,
                    in_=w1h[0:256, 0:H].rearrange("(i p) c -> p i c", p=128))
            xl = x1p.tile([128, 2, BLK + 1], f8, tag=f"xl{kp}", name=f"x1l_0_{kp}")
            x1dma(xl, x1l, kp, 0)
            x1kl0.append(xl)
            # per-pair halves of the quad tiles, issued alongside their
            # k-pair's x tiles so the weight stream never falls behind.
            j, s = kp // 2, 2 * (kp % 2)
            nc.sync.dma_start(
                out=w1hq[j][:, s:s + 2, :],
                in_=w1h[kp * 256:(kp + 1) * 256, :].rearrange(
                    "(i p) c -> p i c", p=128))
            nc.sync.dma_start(
                out=w1lq[j][:, s:s + 2, :],
                in_=w1l[kp * 256:(kp + 1) * 256, :].rearrange(
                    "(i p) c -> p i c", p=128))
            if kp == 1:
                nc.sync.dma_start(
                    out=b1sb, in_=b1s.rearrange("(q p) o -> p q o", p=128))
                for pq in (0, 1):
                    nc.sync.dma_start(
                        out=xt2wh[:, :, 0, pq, 0:1],
                        in_=c2h.rearrange("(a i p) o -> p a i o",
                                          p=128, a=KP2)[:, :, pq, :])
                    nc.sync.dma_start(
                        out=xt2wl[:, :, 0, pq, 0:1],
                        in_=c2l.rearrange("(a i p) o -> p a i o",
                                          p=128, a=KP2)[:, :, pq, :])

        def lhsA(kp, win, q, part, b):
            """[128, 2, 128] k-pair weight view for window win, q-tile q."""
            if kp == 0 and part == 'h' and win == 0:
                if b == 0 and q == 0:
                    return wfirst
                return wAh[:, :, q * 128:(q + 1) * 128]
            quad = (w1hq if part == 'h' else w1lq)[kp // 2]
            s = 2 * (kp % 2)
            c0 = win * H + q * 128
            return quad[:, s:s + 2, c0:c0 + 128]

        for b in range(NBLK):
            if b == 0:
                x1kh, x1kl = x1kh0, x1kl0
            else:
                x1kh, x1kl = [], []
                for kp in range(KP1):
                    xh = x1p.tile([128, 2, BLK + 1], f8, tag=f"xh{kp}",
                                  name=f"x1h_{b}_{kp}")
                    x1dma(xh, x1h, kp, b)
                    x1kh.append(xh)
                    xl = x1p.tile([128, 2, BLK + 1], f8, tag=f"xl{kp}",
                                  name=f"x1l_{b}_{kp}")
                    x1dma(xl, x1l, kp, b)
                    x1kl.append(xl)
            psA = [psp.tile([128, BLK], fp32, tag="mm", name=f"psA_{b}_{q}")
                   for q in range(QT1)]

            def mmA(q, kp, win, p, start, stop, b=b, x1kh=x1kh, x1kl=x1kl,
                    psA=psA):
                wpart = 'h' if p in ('hh', 'lh') else 'l'
                xt = x1kh[kp] if p in ('hh', 'hl') else x1kl[kp]
                nc.tensor.matmul(
                    psA[q], lhsT=lhsA(kp, win, q, wpart, b),
                    rhs=xt[:, :, win:win + BLK],
                    start=start, stop=stop, perf_mode=DR)

            for kp in range(KP1):
                last = (kp == KP1 - 1)
                if not last:
                    if kp == 0 and b == 0:
                        # startup wire race: run every round that needs only
                        # wfirst/wAh/x tiles before the quad-gated ones.
                        rounds = [(0, 'hh'), (0, 'lh'), (1, 'hh'), (1, 'lh'),
                                  (0, 'hl'), (1, 'hl')]
                    else:
                        rounds = [(w, p) for w in (0, 1)
                                  for p in ('hh', 'lh', 'hl')]
                    for win, p in rounds:
                        for q in range(QT1):
                            mmA(q, kp, win, p,
                                start=(kp == 0 and win == 0 and p == 'hh'),
                                stop=False)
                    continue
                # final k-pair: per-q matmuls then the bank-freeing t2 copy;
                # the hi/lo quantization chains run afterwards so all 8 PSUM
                # banks free at t2-copy rate for the next block.
                t2s = []
                for q in range(QT1):
                    for win in (0, 1):
                        for p in ('hh', 'lh', 'hl'):
                            mmA(q, kp, win, p, start=False,
                                stop=(win == 1 and p == 'hl'))
                    t2 = t2p.tile([128, BLK], fp32, tag="t2",
                                  name=f"t2_{b}_{q}")
                    nc.scalar.activation(
                        out=t2, in_=psA[q], func=Act.Identity,
                        bias=b1sb[:, q, :], scale=S2 / (SX * SW1))
                    t2s.append(t2)
                for q in range(QT1):
                    kq, pq = q // 2, q % 2
                    c0, c1 = 1 + b * BLK, 1 + (b + 1) * BLK
                    w0 = min(c1, LP2) - c0   # win0 never stores col L
                    hi0 = xt2wh[:, kq, 0, pq, c0:c0 + w0]
                    nc.scalar.activation(out=hi0, in_=t2s[q][:, 0:w0],
                                         func=Act.Identity)
                    hi1 = xt2wh[:, kq, 1, pq, c0 - 1:c1 - 1]
                    nc.scalar.activation(out=hi1, in_=t2s[q], func=Act.Identity)
                    nc.vector.tensor_sub(out=t2s[q], in0=t2s[q], in1=hi1)
                    nc.scalar.activation(
                        out=xt2wl[:, kq, 1, pq, c0 - 1:c1 - 1],
                        in_=t2s[q], func=Act.Identity)
                    nc.vector.tensor_copy(
                        out=xt2wl[:, kq, 0, pq, c0:c0 + w0],
                        in_=xt2wl[:, kq, 1, pq, c0 - 1:c0 - 1 + w0])

        # ---------------- Phase B: layer 2 + residual + RMSNorm -------------
        # token-major fp8 DoubleRow: lhsT = xt2 hi/lo k-row pairs from the
        # materialized window copies (aligned offsets), rhs = W2 hi/lo pair
        # tiles reusing the W1 quad slots. rowc/out stay bf16; the psum
        # descale folds into xres (host-scaled) and EPS_EFF.
        w2hp, w2lp = [], []
        for kp in range(KP2):
            wh = wpool.tile([128, 2, 2 * D], f8, tag=f"w{kp}", name=f"w2h_{kp}")
            nc.sync.dma_start(
                out=wh, in_=w2h[kp * 256:(kp + 1) * 256, :].rearrange(
                    "(i p) c -> p i c", p=128))
            w2hp.append(wh)
            wl = wpool.tile([128, 2, 2 * D], f8, tag=f"w{4 + kp}", name=f"w2l_{kp}")
            nc.sync.dma_start(
                out=wl, in_=w2l[kp * 256:(kp + 1) * 256, :].rearrange(
                    "(i p) c -> p i c", p=128))
            w2lp.append(wl)

        for j in range(NTT):
            tok0 = j * 128
            # the last tile drains with finer feature slices: a shorter
            # add/square chain between its final matmul and the out DMA.
            fs = FS // 2 if j == NTT - 1 else FS
            nfs = D // fs
            xr = xresp.tile([128, D], bf16, tag="xres", name=f"xres_{j}")
            nc.sync.dma_start(out=xr, in_=xres[tok0:tok0 + 128, :])
            rowc = rowp.tile([128, D], bf16, tag="rowc", name=f"rowc_{j}")
            acc = tmp.tile([128, nfs], fp32, tag="acc", name=f"acc_{j}")
            for q in range(nfs):
                sl = slice(q * fs, (q + 1) * fs)
                ps = psp.tile([128, fs], fp32, tag="mm", name=f"psB_{j}_{q}")
                first = True
                for kp in range(KP2):
                    for win in (0, 1):
                        c0 = win * D + q * fs
                        for p in ('hh', 'lh', 'hl'):
                            xt = xt2wh if p in ('hh', 'hl') else xt2wl
                            wt = (w2hp if p in ('hh', 'lh') else w2lp)[kp]
                            nc.tensor.matmul(
                                ps, lhsT=xt[:, kp, win, :, tok0:tok0 + 128],
                                rhs=wt[:, :, c0:c0 + fs],
                                start=first,
                                stop=(kp == KP2 - 1 and win == 1 and p == 'hl'),
                                perf_mode=DR)
                            first = False
                # rowc = S2*SW2*o3 slice (xres is pre-scaled on the host);
                # partial sum-of-squares right away.
                nc.vector.tensor_add(out=rowc[:, sl], in0=ps, in1=xr[:, sl])
                sq = scr.tile([128, fs], bf16, tag="sq", name=f"sq_{j}_{q}")
                nc.scalar.activation(
                    out=sq, in_=rowc[:, sl],
                    func=Act.Square, accum_out=acc[:, q:q + 1])
            # rstd_eff = 1/sqrt(S/D + EPS*(S2*SW2)^2) absorbs the descale, so
            # rowc * rstd_eff is the final normalized output.
            rstd = tmp.tile([128, 1], fp32, tag="rstd", name=f"rstd_{j}")
            nc.vector.tensor_reduce(
                out=rstd, in_=acc, axis=mybir.AxisListType.X,
                op=mybir.AluOpType.add)
            nc.scalar.activation(
                out=rstd, in_=rstd, func=Act.Sqrt, bias=epssb, scale=1.0 / D)
            nc.vector.reciprocal(out=rstd, in_=rstd)
            for q in range(nfs):
                sl = slice(q * fs, (q + 1) * fs)
                nc.vector.tensor_scalar_mul(
                    out=rowc[:, sl], in0=rowc[:, sl], scalar1=rstd)
                if (q + 1) % (nfs // 2) == 0:
                    h0 = (q + 1 - nfs // 2) * fs
                    nc.sync.dma_start(
                        out=out[tok0:tok0 + 128, h0:(q + 1) * fs],
                        in_=rowc[:, h0:(q + 1) * fs])

    nc.finalize()
    _NC_CACHE["nc"] = nc
    return nc


def _np_reference(inputs, pre_lf_indexs, out_lf_indexs, input_lf_loc, out_lf_loc,
                  inputs_loc, outputs_loc, lf1_caches, lf2_caches,
                  conv1_weight, conv2_weight, conv1_bias, conv2_bias, ln_weight):
    """Generic numpy fallback (only used if the index structure is unexpected)."""
    def fused(x, cache, pre_idx, in_lf_loc, in_loc, out_loc, W):
        bs = pre_idx.shape[0]
        xt = np.zeros((x.shape[0] + bs, x.shape[1]), x.dtype)
        xt[in_loc] = x
        xt[in_lf_loc] = cache[pre_idx]
        c = xt @ W
        h = c.shape[1] // 2
        y = c[:-1, :h] + c[1:, h:]
        return y[out_loc]

    o1 = fused(inputs, lf1_caches, pre_lf_indexs, input_lf_loc,
               inputs_loc, outputs_loc, conv1_weight) + conv1_bias
    o2 = fused(o1, lf2_caches, pre_lf_indexs, input_lf_loc,
               inputs_loc, outputs_loc, conv2_weight) + conv2_bias
    o3 = o2 + inputs
    var = np.mean(o3 * o3, axis=-1, keepdims=True)
    return (o3 / np.sqrt(var + EPS) * ln_weight).astype(np.float32)


def _split8(v, s):
    """hi/lo e4m3 pair representing v*s."""
    e4 = ml_dtypes.float8_e4m3
    hi = np.ascontiguousarray((v * s).astype(e4))
    lo = np.ascontiguousarray((v * s - hi.astype(np.float32)).astype(e4))
    return hi, lo


def kernel(**inputs):
    global LAST_EXEC_NS, LAST_RESULTS
    inp = {k: np.asarray(v) for k, v in inputs.items()}
    x = inp["inputs"].astype(np.float32, copy=False)
    lnw = inp["ln_weight"].astype(np.float32, copy=False)

    s = np.arange(BS, dtype=np.int64)
    j = np.arange(L, dtype=np.int64)
    structured = (
        np.array_equal(inp["inputs_loc"], (s[:, None] * (L + 1) + 1 + j[None, :]).reshape(-1))
        and np.array_equal(inp["outputs_loc"], (s[:, None] * (L + 1) + j[None, :]).reshape(-1))
        and np.array_equal(inp["input_lf_loc"], s * (L + 1))
    )
    if not structured:
        return _np_reference(**inp)

    from concourse.bass_utils import run_bass_kernel_spmd

    nc = _build_bass()

    bf16 = ml_dtypes.bfloat16
    pre_idx = inp["pre_lf_indexs"].astype(np.int64)
    w1hb, w1lb = _split8(inp["conv1_weight"].astype(np.float32), SW1)
    w2hb, w2lb = _split8(inp["conv2_weight"].astype(np.float32), SW2)
    b1f = np.ascontiguousarray(
        (inp["conv1_bias"].astype(np.float32) * S2).reshape(H, 1))
    b2f = inp["conv2_bias"].astype(np.float32)

    in_maps = []
    for sq in range(BS):
        xs = x[sq * L:(sq + 1) * L]                       # [2048, 2048]
        a = np.empty((D, L + 1), np.float32)
        a[:, 0] = inp["lf1_caches"][pre_idx[sq]]
        a[:, 1:] = xs.T
        xh, xl = _split8(a, SX)
        ch, cl = _split8(
            inp["lf2_caches"][pre_idx[sq]].astype(np.float32).reshape(H, 1), S2)
        in_maps.append({
            "x1h": xh, "x1l": xl,
            "xres": np.ascontiguousarray(
                ((xs + b2f[None, :]) * (S2 * SW2)).astype(bf16)),
            "c2h": ch, "c2l": cl,
            "w1h": w1hb, "w1l": w1lb,
            "w2h": w2hb, "w2l": w2lb,
            "b1s": b1f,
        })

    res = run_bass_kernel_spmd(nc, in_maps, list(range(NCORES)), trace=TRACE)
    LAST_EXEC_NS = res.exec_time_ns
    LAST_RESULTS = res
    out = np.concatenate(
        [res.results[i]["out"].astype(np.float32) for i in range(NCORES)], axis=0)
    if not np.all(lnw == 1.0):
        out = out * lnw[None, :]
    return out.astype(np.float32)
